# revision 7
# baseline (speedup 1.0000x reference)
"""Trainium2 Bass kernel for nn_MobileCMUNeXtBlock (8-core SPMD, batch-parallel).

Block: x -> [residual dw3x3 + QuantReLU/fq] x2 -> 1x1 expand(512) -> 1x1
contract(128) -> 3x3 conv(256), each conv followed by per-tensor fake-quant
(global max -> scale).  Per-tensor quantization forces global-max barriers;
cross-core maxes use scalar AllReduce(max) collectives.

Per-core layout: channels (128) on partitions, spatial HxW=16384 on free dim.
- depthwise convs: 9 shifted multiply-accumulate ops on the Vector engine
  (fp32, exact zero-pad semantics via sub-rectangle access patterns)
- 1x1 expand: exact-ish hi/lo bf16 split of the input (K=256 accumulate),
  integer weights in bf16 (exact); two-pass (max pass, then recompute+quant)
- 1x1 contract & 3x3 up conv: integer-in-bf16 matmuls (quantized activations
  are small ints -> exact in bf16; fp32 PSUM accumulation exact)
- rounds: fp32 magic-number trick (adds/subtracts 1.5*2^23) == round-half-even
"""
import sys
sys.path.insert(0, "/opt/trn_rl_repo")

import numpy as np
import ml_dtypes

import concourse.bass as bass
import concourse.bacc as bacc
import concourse.mybir as mybir
from concourse import tile
from concourse.bass_utils import run_bass_kernel_spmd

N_CORES = 8
C = 128
H = W = 128
NPIX = H * W
CO = 256
CMID = 512
CMAGIC = 12582912.0  # 1.5 * 2^23 : fp32 round-half-even magic constant
EPSF = np.float32(1e-8)
R255 = np.float32(1.0) / np.float32(255.0)
R127 = np.float32(1.0) / np.float32(127.0)

dt = mybir.dt
OP = mybir.AluOpType
AX = mybir.AxisListType
AF = mybir.ActivationFunctionType


def _fq_int(w):
    """Replicate reference fq_sym (8 bit) on host in fp32; return (int_grid, scale)."""
    w = np.asarray(w, np.float32)
    m = np.maximum(np.float32(np.abs(w).max()), EPSF).astype(np.float32)
    scale = (m / np.float32(127.0)).astype(np.float32)
    q = np.clip(np.round((w / scale).astype(np.float32)), -127.0, 127.0).astype(np.float32)
    return q, scale


_CACHE = {}


def _build():
    if "nc" in _CACHE:
        return _CACHE
    nc = bacc.Bacc("TRN2", target_bir_lowering=False, debug=False, num_devices=N_CORES)

    x_ap = nc.dram_tensor("x", [C, NPIX], dt.float32, kind="ExternalInput").ap()
    wints_ap = nc.dram_tensor("wints", [128, 512 + 512 + 9 * 256], dt.bfloat16,
                              kind="ExternalInput").ap()
    consts_ap = nc.dram_tensor("consts", [128, 27], dt.float32, kind="ExternalInput").ap()
    out_ap = nc.dram_tensor("out", [CO, NPIX], dt.float32, kind="ExternalOutput").ap()
    zdram = nc.dram_tensor("zstage", [CO, NPIX], dt.float32)

    RG = [list(range(N_CORES))]
    TAPS = [(ky - 1, kx - 1) for ky in range(3) for kx in range(3)]

    with tile.TileContext(nc) as tc:
        outer_cm = tc.tile_pool(name="outer", bufs=1)
        outer = outer_cm.__enter__()
        dram_cm = tc.tile_pool(name="dram", bufs=1, space="DRAM")
        dram = dram_cm.__enter__()

        # ---- tiny helpers -------------------------------------------------
        sc_tiles = {}

        def sc(name):
            if name not in sc_tiles:
                sc_tiles[name] = outer.tile([128, 1], dt.float32, tag="sc_" + name, name="sc_" + name)
            return sc_tiles[name]

        bounce_i = [0]

        def cross_max(local_128x1, cc_tile, col):
            """cross-partition max of [128,1] -> write scalar into cc_tile[0,col]"""
            i = bounce_i[0]
            bounce_i[0] += 1
            d = dram.tile([128, 1], dt.float32, tag=f"bnc{i}", name=f"bnc{i}")
            nc.sync.dma_start(d[:], local_128x1)
            row = outer.tile([1, 128], dt.float32, tag="row", name=f"row{i}")
            nc.sync.dma_start(row[:], d[:].rearrange("p one -> one p"))
            s = outer.tile([1, 1], dt.float32, tag="sca", name=f"sca{i}")
            nc.vector.tensor_reduce(s[:], row[:], axis=AX.X, op=OP.max)
            nc.sync.dma_start(cc_tile[0:1, col:col + 1], s[:])

        def bcast(cc_out_tile, col, name):
            """broadcast cc_out[0,col] -> [128,1] tile with EPS clamp"""
            b = sc(name)
            nc.sync.dma_start(b[:], cc_out_tile[0:1, col:col + 1].partition_broadcast(128))
            nc.vector.tensor_scalar(b[:], b[:], float(EPSF), None, OP.max)
            return b

        def ts(out, in_, s1, s2, op0, op1=None):
            nc.vector.tensor_scalar(out, in_, s1, s2, op0, *( [op1] if op1 else []))

        # ---- persistent small tiles --------------------------------------
        consts = outer.tile([128, 27], dt.float32, tag="consts")
        nc.sync.dma_start(consts[:], consts_ap[:])
        wUV = outer.tile([128, 1024], dt.bfloat16, tag="wUV")
        nc.sync.dma_start(wUV[:], wints_ap[:, 0:1024])

        m7part = outer.tile([128, 128], dt.float32, tag="m7part")
        m8part = outer.tile([128, 32], dt.float32, tag="m8part")
        m9part = outer.tile([128, 64], dt.float32, tag="m9part")

        cc0i = dram.tile([1, 1], dt.float32, tag="cc0i", name="cc0i"); cc0o = dram.tile([1, 1], dt.float32, tag="cc0o", name="cc0o")
        cc1i = dram.tile([1, 1], dt.float32, tag="cc1i", name="cc1i"); cc1o = dram.tile([1, 1], dt.float32, tag="cc1o", name="cc1o")
        cc2i = dram.tile([1, 2], dt.float32, tag="cc2i", name="cc2i"); cc2o = dram.tile([1, 2], dt.float32, tag="cc2o", name="cc2o")
        cc3i = dram.tile([1, 1], dt.float32, tag="cc3i", name="cc3i"); cc3o = dram.tile([1, 1], dt.float32, tag="cc3o", name="cc3o")
        cc4i = dram.tile([1, 1], dt.float32, tag="cc4i", name="cc4i"); cc4o = dram.tile([1, 1], dt.float32, tag="cc4o", name="cc4o")
        cc5i = dram.tile([1, 1], dt.float32, tag="cc5i", name="cc5i"); cc5o = dram.tile([1, 1], dt.float32, tag="cc5o", name="cc5o")

        def allreduce(ci, co_):
            nc.gpsimd.collective_compute("AllReduce", OP.max, replica_groups=RG,
                                         ins=[ci.opt()], outs=[co_.opt()])

        # column indices in consts
        BQ1, BQ2 = 0, 1
        DW1_0, DW2_0 = 2, 11
        BPW1_0, BPW2, BUP_0 = 20, 24, 25

        big_cm = tc.tile_pool(name="big", bufs=1)
        big = big_cm.__enter__()

        def dwconv(dst, src, tap0col):
            """depthwise 3x3, zero padding; dst/src are [128,H,W] f32 tiles."""
            w0 = consts[:, tap0col + 4:tap0col + 5]  # center tap (dy=0,dx=0) index 4
            nc.vector.tensor_scalar(dst[:, :, :], src[:, :, :], w0, None, OP.mult)
            for t, (dy, dx) in enumerate(TAPS):
                if (dy, dx) == (0, 0):
                    continue
                wt = consts[:, tap0col + t:tap0col + t + 1]
                ys, ye = max(0, -dy), H - max(0, dy)
                xs, xe = max(0, -dx), W - max(0, dx)
                o = dst[:, ys:ye, xs:xe]
                i_ = src[:, ys + dy:ye + dy, xs + dx:xe + dx]
                nc.vector.scalar_tensor_tensor(o, i_, wt, o, OP.mult, OP.add)

        # =================== P0/P1: load x, dw1, maxes =====================
        xT = big.tile([128, H, W], dt.float32, tag="A")
        nc.sync.dma_start(xT[:, :, :], x_ap[:].rearrange("c (h w) -> c h w", h=H))

        # m3 = max|x| -> collective 0 (overlaps dw1)
        mx3 = sc("mx3loc")
        nc.vector.tensor_reduce(mx3[:], xT[:, :, :], axis=AX.XY, op=OP.max,
                                apply_absolute_value=True)
        cross_max(mx3[:], cc0i, 0)
        allreduce(cc0i, cc0o)
        m3 = bcast(cc0o, 0, "m3")
        s3 = sc("s3"); a3 = sc("a3")
        ts(s3[:], m3[:], float(R127), None, OP.mult)
        nc.vector.reciprocal(a3[:], s3[:])

        d1 = big.tile([128, H, W], dt.float32, tag="K")
        dwconv(d1, xT, DW1_0)
        # v1 = relu(sw1*d1 + bq1)  (in place)
        sw1, sw2, swp1, swp2, swu = _CACHE["host_scales"]

        nc.scalar.activation(d1[:, :, :], d1[:, :, :], AF.Relu,
                             bias=consts[:, BQ1:BQ1 + 1], scale=1.0)
        mx1 = sc("mx1loc")
        nc.vector.tensor_reduce(mx1[:], d1[:, :, :], axis=AX.XY, op=OP.max)
        cross_max(mx1[:], cc1i, 0)
        allreduce(cc1i, cc1o)
        m1 = bcast(cc1o, 0, "m1")
        s1 = sc("s1"); a1 = sc("a1"); m1q = sc("m1q"); s2 = sc("s2"); rs2 = sc("rs2"); rho = sc("rho")
        ts(s1[:], m1[:], float(R255), None, OP.mult)
        nc.vector.reciprocal(a1[:], s1[:])
        ts(m1q[:], s1[:], 255.0, None, OP.mult)
        ts(s2[:], m1q[:], float(R127), None, OP.mult)
        nc.vector.reciprocal(rs2[:], s2[:])
        nc.vector.tensor_mul(rho[:], s1[:], rs2[:])

        # =================== P2a: r1 = fq(h1) + fq(x) ======================
        qT = big.tile([128, H, W], dt.float32, tag="B")
        # qx3 = round(x*a3) * s3   (a3 ready early)
        ts(qT[:, :, :], xT[:, :, :], a3[:], CMAGIC, OP.mult, OP.add)
        ts(qT[:, :, :], qT[:, :, :], CMAGIC, s3[:], OP.subtract, OP.mult)
        # k1' chain in place on d1 (holds v1)
        ts(d1[:, :, :], d1[:, :, :], a1[:], CMAGIC, OP.mult, OP.add)      # t = v1*a1 + C
        ts(d1[:, :, :], d1[:, :, :], CMAGIC, rho[:], OP.subtract, OP.mult)  # k1*rho
        ts(d1[:, :, :], d1[:, :, :], CMAGIC, CMAGIC, OP.add, OP.subtract)   # k1'
        # r1 = k1'*s2 + qx3   (in place into qT)
        nc.vector.scalar_tensor_tensor(qT[:, :, :], d1[:, :, :], s2[:], qT[:, :, :],
                                       OP.mult, OP.add)

        # =================== P2b: dw2, m4/m6 ==============================
        mx6 = sc("mx6loc")
        nc.vector.tensor_reduce(mx6[:], qT[:, :, :], axis=AX.XY, op=OP.max,
                                apply_absolute_value=True)
        cross_max(mx6[:], cc2i, 1)
        d2 = big.tile([128, H, W], dt.float32, tag="K")
        dwconv(d2, qT, DW2_0)
        nc.scalar.activation(d2[:, :, :], d2[:, :, :], AF.Relu,
                             bias=consts[:, BQ2:BQ2 + 1], scale=1.0)
        mx4 = sc("mx4loc")
        nc.vector.tensor_reduce(mx4[:], d2[:, :, :], axis=AX.XY, op=OP.max)
        cross_max(mx4[:], cc2i, 0)
        allreduce(cc2i, cc2o)
        m4 = bcast(cc2o, 0, "m4"); m6 = bcast(cc2o, 1, "m6")
        s4 = sc("s4"); a4 = sc("a4"); m4q = sc("m4q"); s5 = sc("s5"); rs5 = sc("rs5")
        rho2 = sc("rho2"); s6 = sc("s6"); a6 = sc("a6")
        ts(s4[:], m4[:], float(R255), None, OP.mult)
        nc.vector.reciprocal(a4[:], s4[:])
        ts(m4q[:], s4[:], 255.0, None, OP.mult)
        ts(s5[:], m4q[:], float(R127), None, OP.mult)
        nc.vector.reciprocal(rs5[:], s5[:])
        nc.vector.tensor_mul(rho2[:], s4[:], rs5[:])
        ts(s6[:], m6[:], float(R127), None, OP.mult)
        nc.vector.reciprocal(a6[:], s6[:])

        # =================== P3a: r2 + hi/lo split ========================
        # k2' chain in place on d2 (holds v2)
        ts(d2[:, :, :], d2[:, :, :], a4[:], CMAGIC, OP.mult, OP.add)
        ts(d2[:, :, :], d2[:, :, :], CMAGIC, rho2[:], OP.subtract, OP.mult)
        ts(d2[:, :, :], d2[:, :, :], CMAGIC, CMAGIC, OP.add, OP.subtract)   # k2'
        # j3 = round(r1*a6)*s6 in place on qT
        ts(qT[:, :, :], qT[:, :, :], a6[:], CMAGIC, OP.mult, OP.add)
        ts(qT[:, :, :], qT[:, :, :], CMAGIC, s6[:], OP.subtract, OP.mult)
        r2T = big.tile([128, NPIX], dt.float32, tag="A")
        nc.vector.scalar_tensor_tensor(r2T[:, :], d2[:, :, :].rearrange("c h w -> c (h w)"),
                                       s5[:], qT[:, :, :].rearrange("c h w -> c (h w)"),
                                       OP.mult, OP.add)
        r2h = big.tile([128, NPIX], dt.bfloat16, tag="K")
        nc.vector.tensor_copy(r2h[:, :], r2T[:, :])
        r2l = big.tile([128, NPIX], dt.bfloat16, tag="B")
        nc.vector.tensor_tensor(r2l[:, :], r2T[:, :], r2h[:, :], OP.subtract)

        # =================== P3b: pw1 pass 1 (max only) ====================
        ps1_cm = tc.tile_pool(name="ps1", bufs=3, space="PSUM")
        ps1 = ps1_cm.__enter__()
        p3_cm = tc.tile_pool(name="p3", bufs=6)
        p3 = p3_cm.__enter__()
        NT = NPIX // 512  # 32 n tiles
        for cchunk in range(4):
            lhs = wUV[:, 128 * cchunk:128 * (cchunk + 1)]
            for i in range(NT):
                ps = ps1.tile([128, 512], dt.float32, tag="pw1", name=f"psA_{cchunk}_{i}")
                nc.tensor.matmul(ps[:], lhs, r2h[:, 512 * i:512 * (i + 1)], start=True, stop=False)
                nc.tensor.matmul(ps[:], lhs, r2l[:, 512 * i:512 * (i + 1)], start=False, stop=True)
                nc.vector.tensor_reduce(m7part[:, cchunk * 32 + i:cchunk * 32 + i + 1],
                                        ps[:], axis=AX.X, op=OP.max)
        mx7c = sc("mx7c"); mx7 = sc("mx7loc")
        for cchunk in range(4):
            nc.vector.tensor_reduce(mx7c[:], m7part[:, 32 * cchunk:32 * (cchunk + 1)],
                                    axis=AX.X, op=OP.max)
            nc.scalar.activation(mx7c[:], mx7c[:], AF.Relu,
                                 bias=consts[:, BPW1_0 + cchunk:BPW1_0 + cchunk + 1],
                                 scale=float(swp1))
            if cchunk == 0:
                nc.vector.tensor_copy(mx7[:], mx7c[:])
            else:
                nc.vector.tensor_tensor(mx7[:], mx7[:], mx7c[:], OP.max)
        cross_max(mx7[:], cc3i, 0)
        allreduce(cc3i, cc3o)
        m7 = bcast(cc3o, 0, "m7")
        s7 = sc("s7"); a7 = sc("a7"); al7 = sc("al7"); alc = sc("alc")
        ts(s7[:], m7[:], float(R255), None, OP.mult)
        nc.vector.reciprocal(a7[:], s7[:])
        ts(al7[:], a7[:], float(swp1), None, OP.mult)
        ts(alc[:], s7[:], float(swp2), None, OP.mult)
        bet7 = []
        for cchunk in range(4):
            b_ = sc(f"bet7_{cchunk}")
            nc.vector.tensor_mul(b_[:], consts[:, BPW1_0 + cchunk:BPW1_0 + cchunk + 1], a7[:])
            bet7.append(b_)

        # =================== P3c: pw1 pass 2 + pw2 =========================
        ps2_cm = tc.tile_pool(name="ps2", bufs=2, space="PSUM")
        ps2 = ps2_cm.__enter__()
        z_cT = big.tile([128, NPIX], dt.float32, tag="A")
        for i in range(NT):
            eqs = []
            for cchunk in range(4):
                lhs = wUV[:, 128 * cchunk:128 * (cchunk + 1)]
                ps = ps1.tile([128, 512], dt.float32, tag="pw1", name=f"psA_{cchunk}_{i}")
                nc.tensor.matmul(ps[:], lhs, r2h[:, 512 * i:512 * (i + 1)], start=True, stop=False)
                nc.tensor.matmul(ps[:], lhs, r2l[:, 512 * i:512 * (i + 1)], start=False, stop=True)
                t_ = p3.tile([128, 512], dt.float32, tag="trelu", name=f"tr_{cchunk}_{i}", bufs=2)
                nc.scalar.activation(t_[:], ps[:], AF.Relu, bias=bet7[cchunk][:], scale=al7[:])
                eq = p3.tile([128, 512], dt.bfloat16, tag="eq", name=f"eq_{cchunk}_{i}", bufs=6)
                ts(eq[:], t_[:], CMAGIC, CMAGIC, OP.add, OP.subtract)
                eqs.append(eq)
            ps2t = ps2.tile([128, 512], dt.float32, tag="pw2", name=f"psB_{i}")
            for cchunk in range(4):
                lhsV = wUV[:, 512 + 128 * cchunk:512 + 128 * (cchunk + 1)]
                nc.tensor.matmul(ps2t[:], lhsV, eqs[cchunk][:],
                                 start=(cchunk == 0), stop=(cchunk == 3))
            nc.vector.tensor_reduce(m8part[:, i:i + 1], ps2t[:], axis=AX.X, op=OP.max)
            nc.scalar.activation(z_cT[:, 512 * i:512 * (i + 1)], ps2t[:], AF.Relu,
                                 bias=consts[:, BPW2:BPW2 + 1], scale=alc[:])
        mx8 = sc("mx8loc")
        nc.vector.tensor_reduce(mx8[:], m8part[:, :], axis=AX.X, op=OP.max)
        nc.scalar.activation(mx8[:], mx8[:], AF.Relu, bias=consts[:, BPW2:BPW2 + 1], scale=alc[:])
        cross_max(mx8[:], cc4i, 0)
        allreduce(cc4i, cc4o)
        m8 = bcast(cc4o, 0, "m8")
        s8 = sc("s8"); a8 = sc("a8"); alu = sc("alu")
        ts(s8[:], m8[:], float(R255), None, OP.mult)
        nc.vector.reciprocal(a8[:], s8[:])
        ts(alu[:], s8[:], float(swu), None, OP.mult)

        # =================== P4: cq =======================================
        uT = big.tile([128, NPIX], dt.float32, tag="B")
        ts(uT[:, :], z_cT[:, :], a8[:], CMAGIC, OP.mult, OP.add)
        cqT = big.tile([128, H, W], dt.bfloat16, tag="K")
        ts(cqT[:, :, :].rearrange("c h w -> c (h w)"), uT[:, :], CMAGIC, None, OP.subtract)

        p3_cm.__exit__(None, None, None)
        ps2_cm.__exit__(None, None, None)
        ps1_cm.__exit__(None, None, None)

        # =================== P5: up conv ===================================
        p5_cm = tc.tile_pool(name="p5", bufs=3)
        p5 = p5_cm.__enter__()
        wup = p5.tile([128, 9 * 256], dt.bfloat16, tag="wup", bufs=1)
        nc.sync.dma_start(wup[:], wints_ap[:, 1024:1024 + 9 * 256])
        ps3_cm = tc.tile_pool(name="ps3", bufs=3, space="PSUM")
        ps3 = ps3_cm.__enter__()
        RT = 4  # output rows per tile
        for cchunk in range(2):
            for i in range(NT):
                y0 = i * RT
                ps = ps3.tile([128, RT, W], dt.float32, tag="up", name=f"psU_{cchunk}_{i}")
                # center tap first: full coverage, start=True clears the bank
                lhs_c = wup[:, 256 * 4 + 128 * cchunk: 256 * 4 + 128 * cchunk + 128]
                nc.tensor.matmul(ps[:, :, :], lhs_c,
                                 cqT[:, y0:y0 + RT, :], start=True, stop=False)
                ntap = 0
                for t, (dy, dx) in enumerate(TAPS):
                    if (dy, dx) == (0, 0):
                        continue
                    ntap += 1
                    ys = max(0, -dy, y0) - y0          # local out row start
                    ye = min(H, H - dy, y0 + RT) - y0  # local out row end
                    xs, xe = max(0, -dx), W - max(0, dx)
                    if ye <= ys:
                        continue
                    lhs = wup[:, 256 * t + 128 * cchunk: 256 * t + 128 * cchunk + 128]
                    nc.tensor.matmul(ps[:, ys:ye, xs:xe], lhs,
                                     cqT[:, y0 + ys + dy:y0 + ye + dy, xs + dx:xe + dx],
                                     start=False, stop=(ntap == 8))
                nc.vector.tensor_reduce(m9part[:, cchunk * 32 + i:cchunk * 32 + i + 1],
                                        ps[:, :, :], axis=AX.XY, op=OP.max)
                zt = p5.tile([128, RT * W], dt.float32, tag="zt", name=f"zt_{cchunk}_{i}")
                nc.scalar.activation(zt[:], ps[:, :, :].rearrange("c r w -> c (r w)"),
                                     AF.Relu, bias=consts[:, BUP_0 + cchunk:BUP_0 + cchunk + 1],
                                     scale=alu[:])
                nc.sync.dma_start(
                    zdram.ap()[128 * cchunk:128 * (cchunk + 1), 512 * i:512 * (i + 1)], zt[:])
        mx9c = sc("mx9c"); mx9 = sc("mx9loc")
        for cchunk in range(2):
            nc.vector.tensor_reduce(mx9c[:], m9part[:, 32 * cchunk:32 * (cchunk + 1)],
                                    axis=AX.X, op=OP.max)
            nc.scalar.activation(mx9c[:], mx9c[:], AF.Relu,
                                 bias=consts[:, BUP_0 + cchunk:BUP_0 + cchunk + 1],
                                 scale=alu[:])
            if cchunk == 0:
                nc.vector.tensor_copy(mx9[:], mx9c[:])
            else:
                nc.vector.tensor_tensor(mx9[:], mx9[:], mx9c[:], OP.max)
        cross_max(mx9[:], cc5i, 0)
        allreduce(cc5i, cc5o)
        m9 = bcast(cc5o, 0, "m9")
        s9 = sc("s9"); a9 = sc("a9")
        ts(s9[:], m9[:], float(R255), None, OP.mult)
        nc.vector.reciprocal(a9[:], s9[:])

        ps3_cm.__exit__(None, None, None)
        p5_cm.__exit__(None, None, None)
        big_cm.__exit__(None, None, None)

        # =================== P6: final quantize + store ====================
        p6_cm = tc.tile_pool(name="p6", bufs=3)
        p6 = p6_cm.__enter__()
        FCOL = 2048
        for cchunk in range(2):
            for f in range(NPIX // FCOL):
                ft = p6.tile([128, FCOL], dt.float32, tag="fin", name=f"fin_{cchunk}_{f}")
                nc.sync.dma_start(ft[:], zdram.ap()[128 * cchunk:128 * (cchunk + 1),
                                                    FCOL * f:FCOL * (f + 1)])
                ts(ft[:], ft[:], a9[:], CMAGIC, OP.mult, OP.add)
                ts(ft[:], ft[:], CMAGIC, s9[:], OP.subtract, OP.mult)
                nc.sync.dma_start(out_ap[128 * cchunk:128 * (cchunk + 1),
                                         FCOL * f:FCOL * (f + 1)], ft[:])
        p6_cm.__exit__(None, None, None)
        dram_cm.__exit__(None, None, None)
        outer_cm.__exit__(None, None, None)

    nc.compile()
    _CACHE["nc"] = nc
    return _CACHE


def _prep_host(inputs):
    """Host-side exact weight fake-quant + packing. Returns (wints, consts, scales)."""
    q1, sdw1 = _fq_int(inputs["dw1_w"])     # (128,1,3,3)
    qb1, sb1 = _fq_int(inputs["dw1_b"])
    q2, sdw2 = _fq_int(inputs["dw2_w"])
    qb2, sb2 = _fq_int(inputs["dw2_b"])
    qp1, sp1 = _fq_int(inputs["pw1_w"])     # (512,128,1,1)
    qbp1, sbp1 = _fq_int(inputs["pw1_b"])
    qp2, sp2 = _fq_int(inputs["pw2_w"])     # (128,512,1,1)
    qbp2, sbp2 = _fq_int(inputs["pw2_b"])
    qu, su = _fq_int(inputs["up_w"])        # (256,128,3,3)
    qbu, sbu = _fq_int(inputs["up_b"])

    # consts [128, 27] fp32
    consts = np.zeros((128, 27), np.float32)
    consts[:, 0] = (qb1 * sb1).astype(np.float32)
    consts[:, 1] = (qb2 * sb2).astype(np.float32)
    w1v = (q1 * sdw1).astype(np.float32)    # actual fp32 quantized weight values
    w2v = (q2 * sdw2).astype(np.float32)
    for t in range(9):
        ky, kx = t // 3, t % 3
        consts[:, 2 + t] = w1v[:, 0, ky, kx]
        consts[:, 11 + t] = w2v[:, 0, ky, kx]
    bp1v = (qbp1 * sbp1).astype(np.float32)
    for cchunk in range(4):
        consts[:, 20 + cchunk] = bp1v[128 * cchunk:128 * (cchunk + 1)]
    consts[:, 24] = (qbp2 * sbp2).astype(np.float32)
    bupv = (qbu * sbu).astype(np.float32)
    consts[:, 25] = bupv[0:128]
    consts[:, 26] = bupv[128:256]

    # wints [128, 1024 + 2304] bf16
    wints = np.zeros((128, 512 + 512 + 9 * 256), ml_dtypes.bfloat16)
    wints[:, 0:512] = qp1[:, :, 0, 0].T.astype(ml_dtypes.bfloat16)      # U lhsT [ci, co]
    V = qp2[:, :, 0, 0]                                                  # (128, 512)
    for cchunk in range(4):
        wints[:, 512 + 128 * cchunk:512 + 128 * (cchunk + 1)] = \
            V[:, 128 * cchunk:128 * (cchunk + 1)].T.astype(ml_dtypes.bfloat16)
    for t in range(9):
        ky, kx = t // 3, t % 3
        wints[:, 1024 + 256 * t:1024 + 256 * (t + 1)] = \
            qu[:, :, ky, kx].T.astype(ml_dtypes.bfloat16)
    scales = (float(sdw1), float(sdw2), float(sp1), float(sp2), float(su))
    return wints, consts, scales


def kernel(**inputs):
    wints, consts, scales = _prep_host(inputs)
    if "host_scales" in _CACHE:
        assert _CACHE["host_scales"] == scales, "weight scales changed; rebuild needed"
    _CACHE["host_scales"] = scales
    cache = _build()
    nc = cache["nc"]
    x = np.asarray(inputs["x"], np.float32)  # (8,128,128,128)
    in_maps = [{"x": np.ascontiguousarray(x[b].reshape(C, NPIX)),
                "wints": wints, "consts": consts} for b in range(N_CORES)]
    res = run_bass_kernel_spmd(nc, in_maps, list(range(N_CORES)))
    out = np.stack([res.results[b]["out"].reshape(CO, H, W) for b in range(N_CORES)])
    return out.astype(np.float32)


# revision 8
# speedup vs baseline: 57.5065x; 57.5065x over previous
"""Trainium2 Bass kernel for nn_MobileCMUNeXtBlock (8-core SPMD, batch-parallel).

Block: x -> [residual dw3x3 + QuantReLU/fq] x2 -> 1x1 expand(512) -> 1x1
contract(128) -> 3x3 conv(256), each conv followed by per-tensor fake-quant
(global max -> scale).  Per-tensor quantization forces global-max barriers;
cross-core maxes use scalar AllReduce(max) collectives.

Per-core layout: channels (128) on partitions, spatial HxW=16384 on free dim.
- depthwise convs: 9 shifted multiply-accumulate ops on the Vector engine
  (fp32, exact zero-pad semantics via sub-rectangle access patterns)
- 1x1 expand: exact-ish hi/lo bf16 split of the input (K=256 accumulate),
  integer weights in bf16 (exact); two-pass (max pass, then recompute+quant)
- 1x1 contract & 3x3 up conv: integer-in-bf16 matmuls (quantized activations
  are small ints -> exact in bf16; fp32 PSUM accumulation exact)
- rounds: fp32 magic-number trick (adds/subtracts 1.5*2^23) == round-half-even
"""
import sys
sys.path.insert(0, "/opt/trn_rl_repo")

import numpy as np
import ml_dtypes

import concourse.bass as bass
import concourse.bacc as bacc
import concourse.mybir as mybir
from concourse import tile
from concourse.bass_utils import run_bass_kernel_spmd

N_CORES = 8
C = 128
H = W = 128
NPIX = H * W
CO = 256
CMID = 512
CMAGIC = 12582912.0  # 1.5 * 2^23 : fp32 round-half-even magic constant
EPSF = np.float32(1e-8)
R255 = np.float32(1.0) / np.float32(255.0)
R127 = np.float32(1.0) / np.float32(127.0)

dt = mybir.dt
OP = mybir.AluOpType
AX = mybir.AxisListType
AF = mybir.ActivationFunctionType


def _fq_int(w):
    """Replicate reference fq_sym (8 bit) on host in fp32; return (int_grid, scale)."""
    w = np.asarray(w, np.float32)
    m = np.maximum(np.float32(np.abs(w).max()), EPSF).astype(np.float32)
    scale = (m / np.float32(127.0)).astype(np.float32)
    q = np.clip(np.round((w / scale).astype(np.float32)), -127.0, 127.0).astype(np.float32)
    return q, scale


_CACHE = {}


def _build():
    if "nc" in _CACHE:
        return _CACHE
    nc = bacc.Bacc("TRN2", target_bir_lowering=False, debug=False, num_devices=N_CORES)

    x_ap = nc.dram_tensor("x", [C, NPIX], dt.float32, kind="ExternalInput").ap()
    wints_ap = nc.dram_tensor("wints", [128, 512 + 512 + 9 * 256], dt.bfloat16,
                              kind="ExternalInput").ap()
    consts_ap = nc.dram_tensor("consts", [128, 27], dt.float32, kind="ExternalInput").ap()
    out_ap = nc.dram_tensor("out", [CO, NPIX], dt.float32, kind="ExternalOutput").ap()
    zdram = nc.dram_tensor("zstage", [CO, NPIX], dt.float32)

    RG = [list(range(N_CORES))]
    TAPS = [(ky - 1, kx - 1) for ky in range(3) for kx in range(3)]

    with tile.TileContext(nc) as tc:
        outer_cm = tc.tile_pool(name="outer", bufs=1)
        outer = outer_cm.__enter__()
        dram_cm = tc.tile_pool(name="dram", bufs=1, space="DRAM")
        dram = dram_cm.__enter__()

        # ---- tiny helpers -------------------------------------------------
        sc_tiles = {}

        def sc(name):
            if name not in sc_tiles:
                sc_tiles[name] = outer.tile([128, 1], dt.float32, tag="sc_" + name, name="sc_" + name)
            return sc_tiles[name]

        bounce_i = [0]

        def cross_max(local_128x1, cc_tile, col):
            """cross-partition max of [128,1] -> write scalar into cc_tile[0,col]"""
            i = bounce_i[0]
            bounce_i[0] += 1
            d = dram.tile([128, 1], dt.float32, tag=f"bnc{i}", name=f"bnc{i}")
            nc.sync.dma_start(d[:], local_128x1)
            row = outer.tile([1, 128], dt.float32, tag="row", name=f"row{i}")
            nc.sync.dma_start(row[:], d[:].rearrange("p one -> one p"))
            s = outer.tile([1, 1], dt.float32, tag="sca", name=f"sca{i}")
            nc.vector.tensor_reduce(s[:], row[:], axis=AX.X, op=OP.max)
            nc.sync.dma_start(cc_tile[0:1, col:col + 1], s[:])

        def bcast(cc_out_tile, col, name):
            """broadcast cc_out[0,col] -> [128,1] tile with EPS clamp"""
            b = sc(name)
            nc.sync.dma_start(b[:], cc_out_tile[0:1, col:col + 1].partition_broadcast(128))
            nc.vector.tensor_scalar(b[:], b[:], float(EPSF), None, OP.max)
            return b

        def ts(out, in_, s1, s2, op0, op1=None):
            nc.vector.tensor_scalar(out, in_, s1, s2, op0, *( [op1] if op1 else []))

        # ---- persistent small tiles --------------------------------------
        consts = outer.tile([128, 27], dt.float32, tag="consts")
        nc.sync.dma_start(consts[:], consts_ap[:])
        wUV = outer.tile([128, 1024], dt.bfloat16, tag="wUV")
        nc.sync.dma_start(wUV[:], wints_ap[:, 0:1024])

        m7part = outer.tile([128, 128], dt.float32, tag="m7part")
        m8part = outer.tile([128, 32], dt.float32, tag="m8part")
        m9part = outer.tile([128, 64], dt.float32, tag="m9part")

        cc0i = dram.tile([1, 1], dt.float32, tag="cc0i", name="cc0i"); cc0o = dram.tile([1, 1], dt.float32, tag="cc0o", name="cc0o")
        cc1i = dram.tile([1, 1], dt.float32, tag="cc1i", name="cc1i"); cc1o = dram.tile([1, 1], dt.float32, tag="cc1o", name="cc1o")
        cc2i = dram.tile([1, 2], dt.float32, tag="cc2i", name="cc2i"); cc2o = dram.tile([1, 2], dt.float32, tag="cc2o", name="cc2o")
        cc3i = dram.tile([1, 1], dt.float32, tag="cc3i", name="cc3i"); cc3o = dram.tile([1, 1], dt.float32, tag="cc3o", name="cc3o")
        cc4i = dram.tile([1, 1], dt.float32, tag="cc4i", name="cc4i"); cc4o = dram.tile([1, 1], dt.float32, tag="cc4o", name="cc4o")
        cc5i = dram.tile([1, 1], dt.float32, tag="cc5i", name="cc5i"); cc5o = dram.tile([1, 1], dt.float32, tag="cc5o", name="cc5o")

        def allreduce(ci, co_):
            nc.gpsimd.collective_compute("AllReduce", OP.max, replica_groups=RG,
                                         ins=[ci.opt()], outs=[co_.opt()])

        # column indices in consts
        BQ1, BQ2 = 0, 1
        DW1_0, DW2_0 = 2, 11
        BPW1_0, BPW2, BUP_0 = 20, 24, 25

        big_cm = tc.tile_pool(name="big", bufs=1)
        big = big_cm.__enter__()

        def dwconv(dst, src, tap0col):
            """depthwise 3x3, zero padding; dst/src are [128,H,W] f32 tiles."""
            w0 = consts[:, tap0col + 4:tap0col + 5]  # center tap (dy=0,dx=0) index 4
            nc.vector.tensor_scalar(dst[:, :, :], src[:, :, :], w0, None, OP.mult)
            for t, (dy, dx) in enumerate(TAPS):
                if (dy, dx) == (0, 0):
                    continue
                wt = consts[:, tap0col + t:tap0col + t + 1]
                ys, ye = max(0, -dy), H - max(0, dy)
                xs, xe = max(0, -dx), W - max(0, dx)
                o = dst[:, ys:ye, xs:xe]
                i_ = src[:, ys + dy:ye + dy, xs + dx:xe + dx]
                nc.vector.scalar_tensor_tensor(o, i_, wt, o, OP.mult, OP.add)

        # =================== P0/P1: load x, dw1, maxes =====================
        xT = big.tile([128, H, W], dt.float32, tag="A")
        nc.sync.dma_start(xT[:, :, :], x_ap[:].rearrange("c (h w) -> c h w", h=H))

        # m3 = max|x| -> collective 0 (overlaps dw1)
        mx3 = sc("mx3loc")
        nc.vector.tensor_reduce(mx3[:], xT[:, :, :], axis=AX.XY, op=OP.max,
                                apply_absolute_value=True)
        cross_max(mx3[:], cc0i, 0)
        allreduce(cc0i, cc0o)
        m3 = bcast(cc0o, 0, "m3")
        s3 = sc("s3"); a3 = sc("a3")
        ts(s3[:], m3[:], float(R127), None, OP.mult)
        nc.vector.reciprocal(a3[:], s3[:])

        d1 = big.tile([128, H, W], dt.float32, tag="K")
        dwconv(d1, xT, DW1_0)
        # v1 = relu(sw1*d1 + bq1)  (in place)
        sw1, sw2, swp1, swp2, swu = _CACHE["host_scales"]

        nc.scalar.activation(d1[:, :, :], d1[:, :, :], AF.Relu,
                             bias=consts[:, BQ1:BQ1 + 1], scale=1.0)
        mx1 = sc("mx1loc")
        nc.vector.tensor_reduce(mx1[:], d1[:, :, :], axis=AX.XY, op=OP.max)
        cross_max(mx1[:], cc1i, 0)
        allreduce(cc1i, cc1o)
        m1 = bcast(cc1o, 0, "m1")
        s1 = sc("s1"); a1 = sc("a1"); m1q = sc("m1q"); s2 = sc("s2"); rs2 = sc("rs2"); rho = sc("rho")
        ts(s1[:], m1[:], float(R255), None, OP.mult)
        nc.vector.reciprocal(a1[:], s1[:])
        ts(m1q[:], s1[:], 255.0, None, OP.mult)
        ts(s2[:], m1q[:], float(R127), None, OP.mult)
        nc.vector.reciprocal(rs2[:], s2[:])
        nc.vector.tensor_mul(rho[:], s1[:], rs2[:])

        # =================== P2a: r1 = fq(h1) + fq(x) ======================
        qT = big.tile([128, H, W], dt.float32, tag="B")
        # qx3 = round(x*a3) * s3   (a3 ready early)
        ts(qT[:, :, :], xT[:, :, :], a3[:], CMAGIC, OP.mult, OP.add)
        ts(qT[:, :, :], qT[:, :, :], CMAGIC, s3[:], OP.subtract, OP.mult)
        # k1' chain in place on d1 (holds v1)
        ts(d1[:, :, :], d1[:, :, :], a1[:], CMAGIC, OP.mult, OP.add)      # t = v1*a1 + C
        ts(d1[:, :, :], d1[:, :, :], CMAGIC, rho[:], OP.subtract, OP.mult)  # k1*rho
        ts(d1[:, :, :], d1[:, :, :], CMAGIC, CMAGIC, OP.add, OP.subtract)   # k1'
        # r1 = k1'*s2 + qx3   (in place into qT)
        nc.vector.scalar_tensor_tensor(qT[:, :, :], d1[:, :, :], s2[:], qT[:, :, :],
                                       OP.mult, OP.add)

        # =================== P2b: dw2, m4/m6 ==============================
        mx6 = sc("mx6loc")
        nc.vector.tensor_reduce(mx6[:], qT[:, :, :], axis=AX.XY, op=OP.max,
                                apply_absolute_value=True)
        cross_max(mx6[:], cc2i, 1)
        d2 = big.tile([128, H, W], dt.float32, tag="K")
        dwconv(d2, qT, DW2_0)
        nc.scalar.activation(d2[:, :, :], d2[:, :, :], AF.Relu,
                             bias=consts[:, BQ2:BQ2 + 1], scale=1.0)
        mx4 = sc("mx4loc")
        nc.vector.tensor_reduce(mx4[:], d2[:, :, :], axis=AX.XY, op=OP.max)
        cross_max(mx4[:], cc2i, 0)
        allreduce(cc2i, cc2o)
        m4 = bcast(cc2o, 0, "m4"); m6 = bcast(cc2o, 1, "m6")
        s4 = sc("s4"); a4 = sc("a4"); m4q = sc("m4q"); s5 = sc("s5"); rs5 = sc("rs5")
        rho2 = sc("rho2"); s6 = sc("s6"); a6 = sc("a6")
        ts(s4[:], m4[:], float(R255), None, OP.mult)
        nc.vector.reciprocal(a4[:], s4[:])
        ts(m4q[:], s4[:], 255.0, None, OP.mult)
        ts(s5[:], m4q[:], float(R127), None, OP.mult)
        nc.vector.reciprocal(rs5[:], s5[:])
        nc.vector.tensor_mul(rho2[:], s4[:], rs5[:])
        ts(s6[:], m6[:], float(R127), None, OP.mult)
        nc.vector.reciprocal(a6[:], s6[:])

        # =================== P3a: r2 + hi/lo split ========================
        # k2' chain in place on d2 (holds v2)
        ts(d2[:, :, :], d2[:, :, :], a4[:], CMAGIC, OP.mult, OP.add)
        ts(d2[:, :, :], d2[:, :, :], CMAGIC, rho2[:], OP.subtract, OP.mult)
        ts(d2[:, :, :], d2[:, :, :], CMAGIC, CMAGIC, OP.add, OP.subtract)   # k2'
        # j3 = round(r1*a6)*s6 in place on qT
        ts(qT[:, :, :], qT[:, :, :], a6[:], CMAGIC, OP.mult, OP.add)
        ts(qT[:, :, :], qT[:, :, :], CMAGIC, s6[:], OP.subtract, OP.mult)
        r2T = big.tile([128, NPIX], dt.float32, tag="A")
        nc.vector.scalar_tensor_tensor(r2T[:, :], d2[:, :, :].rearrange("c h w -> c (h w)"),
                                       s5[:], qT[:, :, :].rearrange("c h w -> c (h w)"),
                                       OP.mult, OP.add)
        r2h = big.tile([128, NPIX], dt.bfloat16, tag="K")
        nc.vector.tensor_copy(r2h[:, :], r2T[:, :])
        r2l = big.tile([128, NPIX], dt.bfloat16, tag="B")
        nc.vector.tensor_tensor(r2l[:, :], r2T[:, :], r2h[:, :], OP.subtract)

        # =================== P3b: pw1 pass 1 (max only) ====================
        ps1_cm = tc.tile_pool(name="ps1", bufs=3, space="PSUM")
        ps1 = ps1_cm.__enter__()
        p3_cm = tc.tile_pool(name="p3", bufs=6)
        p3 = p3_cm.__enter__()
        NT = NPIX // 512  # 32 n tiles
        for cchunk in range(4):
            lhs = wUV[:, 128 * cchunk:128 * (cchunk + 1)]
            for i in range(NT):
                ps = ps1.tile([128, 512], dt.float32, tag="pw1", name=f"psA_{cchunk}_{i}")
                nc.tensor.matmul(ps[:], lhs, r2h[:, 512 * i:512 * (i + 1)], start=True, stop=False)
                nc.tensor.matmul(ps[:], lhs, r2l[:, 512 * i:512 * (i + 1)], start=False, stop=True)
                nc.vector.tensor_reduce(m7part[:, cchunk * 32 + i:cchunk * 32 + i + 1],
                                        ps[:], axis=AX.X, op=OP.max)
        mx7c = sc("mx7c"); mx7 = sc("mx7loc")
        for cchunk in range(4):
            nc.vector.tensor_reduce(mx7c[:], m7part[:, 32 * cchunk:32 * (cchunk + 1)],
                                    axis=AX.X, op=OP.max)
            nc.scalar.activation(mx7c[:], mx7c[:], AF.Relu,
                                 bias=consts[:, BPW1_0 + cchunk:BPW1_0 + cchunk + 1],
                                 scale=float(swp1))
            if cchunk == 0:
                nc.vector.tensor_copy(mx7[:], mx7c[:])
            else:
                nc.vector.tensor_tensor(mx7[:], mx7[:], mx7c[:], OP.max)
        cross_max(mx7[:], cc3i, 0)
        allreduce(cc3i, cc3o)
        m7 = bcast(cc3o, 0, "m7")
        s7 = sc("s7"); a7 = sc("a7"); al7 = sc("al7"); alc = sc("alc")
        ts(s7[:], m7[:], float(R255), None, OP.mult)
        nc.vector.reciprocal(a7[:], s7[:])
        ts(al7[:], a7[:], float(swp1), None, OP.mult)
        ts(alc[:], s7[:], float(swp2), None, OP.mult)
        bet7 = []
        for cchunk in range(4):
            b_ = sc(f"bet7_{cchunk}")
            nc.vector.tensor_mul(b_[:], consts[:, BPW1_0 + cchunk:BPW1_0 + cchunk + 1], a7[:])
            bet7.append(b_)

        # =================== P3c: pw1 pass 2 + pw2 =========================
        ps2_cm = tc.tile_pool(name="ps2", bufs=2, space="PSUM")
        ps2 = ps2_cm.__enter__()
        z_cT = big.tile([128, NPIX], dt.float32, tag="A")
        for i in range(NT):
            eqs = []
            for cchunk in range(4):
                lhs = wUV[:, 128 * cchunk:128 * (cchunk + 1)]
                ps = ps1.tile([128, 512], dt.float32, tag="pw1", name=f"psA_{cchunk}_{i}")
                nc.tensor.matmul(ps[:], lhs, r2h[:, 512 * i:512 * (i + 1)], start=True, stop=False)
                nc.tensor.matmul(ps[:], lhs, r2l[:, 512 * i:512 * (i + 1)], start=False, stop=True)
                t_ = p3.tile([128, 512], dt.float32, tag="trelu", name=f"tr_{cchunk}_{i}", bufs=2)
                nc.scalar.activation(t_[:], ps[:], AF.Relu, bias=bet7[cchunk][:], scale=al7[:])
                eq = p3.tile([128, 512], dt.bfloat16, tag="eq", name=f"eq_{cchunk}_{i}", bufs=6)
                ts(eq[:], t_[:], CMAGIC, CMAGIC, OP.add, OP.subtract)
                eqs.append(eq)
            ps2t = ps2.tile([128, 512], dt.float32, tag="pw2", name=f"psB_{i}")
            for cchunk in range(4):
                lhsV = wUV[:, 512 + 128 * cchunk:512 + 128 * (cchunk + 1)]
                nc.tensor.matmul(ps2t[:], lhsV, eqs[cchunk][:],
                                 start=(cchunk == 0), stop=(cchunk == 3))
            nc.vector.tensor_reduce(m8part[:, i:i + 1], ps2t[:], axis=AX.X, op=OP.max)
            nc.scalar.activation(z_cT[:, 512 * i:512 * (i + 1)], ps2t[:], AF.Relu,
                                 bias=consts[:, BPW2:BPW2 + 1], scale=alc[:])
        mx8 = sc("mx8loc")
        nc.vector.tensor_reduce(mx8[:], m8part[:, :], axis=AX.X, op=OP.max)
        nc.scalar.activation(mx8[:], mx8[:], AF.Relu, bias=consts[:, BPW2:BPW2 + 1], scale=alc[:])
        cross_max(mx8[:], cc4i, 0)
        allreduce(cc4i, cc4o)
        m8 = bcast(cc4o, 0, "m8")
        s8 = sc("s8"); a8 = sc("a8"); alu = sc("alu")
        ts(s8[:], m8[:], float(R255), None, OP.mult)
        nc.vector.reciprocal(a8[:], s8[:])
        ts(alu[:], s8[:], float(swu), None, OP.mult)

        # =================== P4: cq =======================================
        uT = big.tile([128, NPIX], dt.float32, tag="B")
        ts(uT[:, :], z_cT[:, :], a8[:], CMAGIC, OP.mult, OP.add)
        cqT = big.tile([128, H, W], dt.bfloat16, tag="K")
        ts(cqT[:, :, :].rearrange("c h w -> c (h w)"), uT[:, :], CMAGIC, None, OP.subtract)

        p3_cm.__exit__(None, None, None)
        ps2_cm.__exit__(None, None, None)
        ps1_cm.__exit__(None, None, None)

        # =================== P5: up conv ===================================
        p5_cm = tc.tile_pool(name="p5", bufs=3)
        p5 = p5_cm.__enter__()
        wup = p5.tile([128, 9 * 256], dt.bfloat16, tag="wup", bufs=1)
        nc.sync.dma_start(wup[:], wints_ap[:, 1024:1024 + 9 * 256])
        ps3_cm = tc.tile_pool(name="ps3", bufs=3, space="PSUM")
        ps3 = ps3_cm.__enter__()
        RT = 4  # output rows per tile
        for cchunk in range(2):
            for i in range(NT):
                y0 = i * RT
                ps = ps3.tile([128, RT, W], dt.float32, tag="up", name=f"psU_{cchunk}_{i}")
                # center tap first: full coverage, start=True clears the bank
                lhs_c = wup[:, 256 * 4 + 128 * cchunk: 256 * 4 + 128 * cchunk + 128]
                nc.tensor.matmul(ps[:, :, :], lhs_c,
                                 cqT[:, y0:y0 + RT, :], start=True, stop=False)
                ntap = 0
                for t, (dy, dx) in enumerate(TAPS):
                    if (dy, dx) == (0, 0):
                        continue
                    ntap += 1
                    ys = max(0, -dy, y0) - y0          # local out row start
                    ye = min(H, H - dy, y0 + RT) - y0  # local out row end
                    xs, xe = max(0, -dx), W - max(0, dx)
                    if ye <= ys:
                        continue
                    lhs = wup[:, 256 * t + 128 * cchunk: 256 * t + 128 * cchunk + 128]
                    nc.tensor.matmul(ps[:, ys:ye, xs:xe], lhs,
                                     cqT[:, y0 + ys + dy:y0 + ye + dy, xs + dx:xe + dx],
                                     start=False, stop=(ntap == 8))
                nc.vector.tensor_reduce(m9part[:, cchunk * 32 + i:cchunk * 32 + i + 1],
                                        ps[:, :, :], axis=AX.XY, op=OP.max)
                zt = p5.tile([128, RT * W], dt.float32, tag="zt", name=f"zt_{cchunk}_{i}")
                nc.scalar.activation(zt[:], ps[:, :, :].rearrange("c r w -> c (r w)"),
                                     AF.Relu, bias=consts[:, BUP_0 + cchunk:BUP_0 + cchunk + 1],
                                     scale=alu[:])
                nc.sync.dma_start(
                    zdram.ap()[128 * cchunk:128 * (cchunk + 1), 512 * i:512 * (i + 1)], zt[:])
        mx9c = sc("mx9c"); mx9 = sc("mx9loc")
        for cchunk in range(2):
            nc.vector.tensor_reduce(mx9c[:], m9part[:, 32 * cchunk:32 * (cchunk + 1)],
                                    axis=AX.X, op=OP.max)
            nc.scalar.activation(mx9c[:], mx9c[:], AF.Relu,
                                 bias=consts[:, BUP_0 + cchunk:BUP_0 + cchunk + 1],
                                 scale=alu[:])
            if cchunk == 0:
                nc.vector.tensor_copy(mx9[:], mx9c[:])
            else:
                nc.vector.tensor_tensor(mx9[:], mx9[:], mx9c[:], OP.max)
        cross_max(mx9[:], cc5i, 0)
        allreduce(cc5i, cc5o)
        m9 = bcast(cc5o, 0, "m9")
        s9 = sc("s9"); a9 = sc("a9")
        ts(s9[:], m9[:], float(R255), None, OP.mult)
        nc.vector.reciprocal(a9[:], s9[:])

        ps3_cm.__exit__(None, None, None)
        p5_cm.__exit__(None, None, None)
        big_cm.__exit__(None, None, None)

        # =================== P6: final quantize + store ====================
        p6_cm = tc.tile_pool(name="p6", bufs=3)
        p6 = p6_cm.__enter__()
        FCOL = 2048
        for cchunk in range(2):
            for f in range(NPIX // FCOL):
                ft = p6.tile([128, FCOL], dt.float32, tag="fin", name=f"fin_{cchunk}_{f}")
                nc.sync.dma_start(ft[:], zdram.ap()[128 * cchunk:128 * (cchunk + 1),
                                                    FCOL * f:FCOL * (f + 1)])
                ts(ft[:], ft[:], a9[:], CMAGIC, OP.mult, OP.add)
                ts(ft[:], ft[:], CMAGIC, s9[:], OP.subtract, OP.mult)
                nc.sync.dma_start(out_ap[128 * cchunk:128 * (cchunk + 1),
                                         FCOL * f:FCOL * (f + 1)], ft[:])
        p6_cm.__exit__(None, None, None)
        dram_cm.__exit__(None, None, None)
        outer_cm.__exit__(None, None, None)

    nc.compile()
    _CACHE["nc"] = nc
    return _CACHE


def _prep_host(inputs):
    """Host-side exact weight fake-quant + packing. Returns (wints, consts, scales)."""
    q1, sdw1 = _fq_int(inputs["dw1_w"])     # (128,1,3,3)
    qb1, sb1 = _fq_int(inputs["dw1_b"])
    q2, sdw2 = _fq_int(inputs["dw2_w"])
    qb2, sb2 = _fq_int(inputs["dw2_b"])
    qp1, sp1 = _fq_int(inputs["pw1_w"])     # (512,128,1,1)
    qbp1, sbp1 = _fq_int(inputs["pw1_b"])
    qp2, sp2 = _fq_int(inputs["pw2_w"])     # (128,512,1,1)
    qbp2, sbp2 = _fq_int(inputs["pw2_b"])
    qu, su = _fq_int(inputs["up_w"])        # (256,128,3,3)
    qbu, sbu = _fq_int(inputs["up_b"])

    # consts [128, 27] fp32
    consts = np.zeros((128, 27), np.float32)
    consts[:, 0] = (qb1 * sb1).astype(np.float32)
    consts[:, 1] = (qb2 * sb2).astype(np.float32)
    w1v = (q1 * sdw1).astype(np.float32)    # actual fp32 quantized weight values
    w2v = (q2 * sdw2).astype(np.float32)
    for t in range(9):
        ky, kx = t // 3, t % 3
        consts[:, 2 + t] = w1v[:, 0, ky, kx]
        consts[:, 11 + t] = w2v[:, 0, ky, kx]
    bp1v = (qbp1 * sbp1).astype(np.float32)
    for cchunk in range(4):
        consts[:, 20 + cchunk] = bp1v[128 * cchunk:128 * (cchunk + 1)]
    consts[:, 24] = (qbp2 * sbp2).astype(np.float32)
    bupv = (qbu * sbu).astype(np.float32)
    consts[:, 25] = bupv[0:128]
    consts[:, 26] = bupv[128:256]

    # wints [128, 1024 + 2304] bf16
    wints = np.zeros((128, 512 + 512 + 9 * 256), ml_dtypes.bfloat16)
    wints[:, 0:512] = qp1[:, :, 0, 0].T.astype(ml_dtypes.bfloat16)      # U lhsT [ci, co]
    V = qp2[:, :, 0, 0]                                                  # (128, 512)
    for cchunk in range(4):
        wints[:, 512 + 128 * cchunk:512 + 128 * (cchunk + 1)] = \
            V[:, 128 * cchunk:128 * (cchunk + 1)].T.astype(ml_dtypes.bfloat16)
    for t in range(9):
        ky, kx = t // 3, t % 3
        wints[:, 1024 + 256 * t:1024 + 256 * (t + 1)] = \
            qu[:, :, ky, kx].T.astype(ml_dtypes.bfloat16)
    scales = (float(sdw1), float(sdw2), float(sp1), float(sp2), float(su))
    return wints, consts, scales


def _get_runner():
    """Persistent jitted 8-core executor (mirrors bass2jax.run_bass_via_pjrt,
    but caches the jitted callable so repeat calls don't re-trace)."""
    if "runner" in _CACHE:
        return _CACHE["runner"]
    import jax
    from jax.sharding import Mesh, PartitionSpec
    from jax.experimental.shard_map import shard_map
    from concourse import bass2jax

    nc = _CACHE["nc"]
    bass2jax.install_neuronx_cc_hook()
    partition_name = nc.partition_id_tensor.name if nc.partition_id_tensor else None
    in_names, out_names, out_avals, zero_outs = [], [], [], []
    for alloc in nc.m.functions[0].allocations:
        if not isinstance(alloc, mybir.MemoryLocationSet):
            continue
        name = alloc.memorylocations[0].name
        if alloc.kind == "ExternalInput":
            if name != partition_name:
                in_names.append(name)
        elif alloc.kind == "ExternalOutput":
            shape = tuple(alloc.tensor_shape)
            dtype = mybir.dt.np(alloc.dtype)
            out_names.append(name)
            out_avals.append(jax.core.ShapedArray(shape, dtype))
            zero_outs.append(np.zeros(shape, dtype))
    n_params = len(in_names)
    all_in_names = list(in_names) + list(out_names)
    if partition_name is not None:
        all_in_names.append(partition_name)

    def _body(*args):
        operands = list(args)
        if partition_name is not None:
            operands.append(bass2jax.partition_id_tensor())
        outs = bass2jax._bass_exec_p.bind(
            *operands,
            out_avals=tuple(out_avals),
            in_names=tuple(all_in_names),
            out_names=tuple(out_names),
            lowering_input_output_aliases=(),
            sim_require_finite=True,
            sim_require_nnan=True,
            nc=nc,
        )
        return tuple(outs)

    devices = jax.devices()[:N_CORES]
    mesh = Mesh(np.asarray(devices), ("core",))
    n_outs = len(out_names)
    in_specs = (PartitionSpec("core"),) * (n_params + n_outs)
    out_specs = (PartitionSpec("core"),) * n_outs
    sharded = jax.jit(
        shard_map(_body, mesh=mesh, in_specs=in_specs, out_specs=out_specs,
                  check_rep=False),
        keep_unused=True,
    )
    concat_zeros = [
        jax.device_put(np.zeros((N_CORES * z.shape[0], *z.shape[1:]), z.dtype))
        for z in zero_outs
    ]

    def run(concat_inputs):
        """concat_inputs: dict name -> (8*dim0, ...) arrays (np or jax)."""
        args = [concat_inputs[n] for n in in_names]
        outs = sharded(*args, *concat_zeros)
        return {name: outs[i] for i, name in enumerate(out_names)}

    _CACHE["runner"] = (run, in_names, out_names)
    return _CACHE["runner"]


def _prep_concat(inputs):
    wints, consts, scales = _prep_host(inputs)
    if "host_scales" in _CACHE:
        assert _CACHE["host_scales"] == scales, "weight scales changed; rebuild needed"
    _CACHE["host_scales"] = scales
    _build()
    x = np.asarray(inputs["x"], np.float32)
    concat = {
        "x": np.ascontiguousarray(x.reshape(N_CORES * C, NPIX)),
        "wints": np.concatenate([wints] * N_CORES, axis=0),
        "consts": np.concatenate([consts] * N_CORES, axis=0),
    }
    return concat


def kernel(**inputs):
    concat = _prep_concat(inputs)
    run, in_names, out_names = _get_runner()
    outs = run(concat)
    out = np.asarray(outs["out"]).reshape(N_CORES, CO, H, W)
    return out.astype(np.float32)


# revision 12
# speedup vs baseline: 58.4234x; 1.0159x over previous
"""Trainium2 Bass kernel for nn_MobileCMUNeXtBlock (8-core SPMD, batch-parallel).

Block: x -> [residual dw3x3 + QuantReLU/fq] x2 -> 1x1 expand(512) -> 1x1
contract(128) -> 3x3 conv(256), each conv followed by per-tensor fake-quant
(global max -> scale).  Per-tensor quantization forces global-max barriers;
cross-core maxes use scalar AllReduce(max) collectives.

Per-core layout: channels (128) on partitions, spatial HxW=16384 on free dim.
- depthwise convs: 9 shifted multiply-accumulate ops on the Vector engine
  (fp32, exact zero-pad semantics via sub-rectangle access patterns)
- 1x1 expand: exact-ish hi/lo bf16 split of the input (K=256 accumulate),
  integer weights in bf16 (exact); two-pass (max pass, then recompute+quant)
- 1x1 contract & 3x3 up conv: integer-in-bf16 matmuls (quantized activations
  are small ints -> exact in bf16; fp32 PSUM accumulation exact)
- rounds: fp32 magic-number trick (adds/subtracts 1.5*2^23) == round-half-even
"""
import sys
sys.path.insert(0, "/opt/trn_rl_repo")

import numpy as np
import ml_dtypes

import concourse.bass as bass
import concourse.bacc as bacc
import concourse.mybir as mybir
from concourse import tile
from concourse.bass_utils import run_bass_kernel_spmd

N_CORES = 8
C = 128
H = W = 128
NPIX = H * W
CO = 256
CMID = 512
CMAGIC = 12582912.0  # 1.5 * 2^23 : fp32 round-half-even magic constant
EPSF = np.float32(1e-8)
R255 = np.float32(1.0) / np.float32(255.0)
R127 = np.float32(1.0) / np.float32(127.0)

dt = mybir.dt
OP = mybir.AluOpType
AX = mybir.AxisListType
AF = mybir.ActivationFunctionType


def _fq_int(w):
    """Replicate reference fq_sym (8 bit) on host in fp32; return (int_grid, scale)."""
    w = np.asarray(w, np.float32)
    m = np.maximum(np.float32(np.abs(w).max()), EPSF).astype(np.float32)
    scale = (m / np.float32(127.0)).astype(np.float32)
    q = np.clip(np.round((w / scale).astype(np.float32)), -127.0, 127.0).astype(np.float32)
    return q, scale


_CACHE = {}


def _build(sim1=False):
    if "nc" in _CACHE:
        return _CACHE
    ndev = 1 if sim1 else N_CORES
    nc = bacc.Bacc("TRN2", target_bir_lowering=False, debug=False, num_devices=ndev)

    x_ap = nc.dram_tensor("x", [C, NPIX], dt.float32, kind="ExternalInput").ap()
    wints_ap = nc.dram_tensor("wints", [128, 512 + 512 + 9 * 256 + 2 * 9 * 128], dt.bfloat16,
                              kind="ExternalInput").ap()
    consts_ap = nc.dram_tensor("consts", [128, 28], dt.float32, kind="ExternalInput").ap()
    out_ap = nc.dram_tensor("out", [CO, NPIX], dt.float32, kind="ExternalOutput").ap()
    zdram = nc.dram_tensor("zstage", [CO, NPIX], dt.float32)

    RG = [list(range(N_CORES))]
    TAPS = [(ky - 1, kx - 1) for ky in range(3) for kx in range(3)]

    with tile.TileContext(nc) as tc:
        outer_cm = tc.tile_pool(name="outer", bufs=1)
        outer = outer_cm.__enter__()
        dram_cm = tc.tile_pool(name="dram", bufs=1, space="DRAM")
        dram = dram_cm.__enter__()

        # ---- tiny helpers -------------------------------------------------
        sc_tiles = {}

        def sc(name):
            if name not in sc_tiles:
                sc_tiles[name] = outer.tile([128, 1], dt.float32, tag="sc_" + name, name="sc_" + name)
            return sc_tiles[name]

        bounce_i = [0]

        def cross_max(local_128x1, cc_tile, col):
            """cross-partition max of [128,1] -> write scalar into cc_tile[0,col]"""
            i = bounce_i[0]
            bounce_i[0] += 1
            d = dram.tile([128, 1], dt.float32, tag=f"bnc{i}", name=f"bnc{i}")
            nc.sync.dma_start(d[:], local_128x1)
            row = outer.tile([1, 128], dt.float32, tag="row", name=f"row{i}")
            nc.sync.dma_start(row[:], d[:].rearrange("p one -> one p"))
            s = outer.tile([1, 1], dt.float32, tag="sca", name=f"sca{i}")
            nc.vector.tensor_reduce(s[:], row[:], axis=AX.X, op=OP.max)
            nc.sync.dma_start(cc_tile[0:1, col:col + 1], s[:])

        def bcast(cc_out_tile, col, name):
            """broadcast cc_out[0,col] -> [128,1] tile with EPS clamp"""
            b = sc(name)
            nc.sync.dma_start(b[:], cc_out_tile[0:1, col:col + 1].partition_broadcast(128))
            nc.vector.tensor_scalar(b[:], b[:], float(EPSF), None, OP.max)
            return b

        def ts(out, in_, s1, s2, op0, op1=None):
            nc.vector.tensor_scalar(out, in_, s1, s2, op0, *( [op1] if op1 else []))

        # ---- persistent small tiles --------------------------------------
        consts = outer.tile([128, 28], dt.float32, tag="consts")
        nc.sync.dma_start(consts[:], consts_ap[:])
        wUV = outer.tile([128, 1024], dt.bfloat16, tag="wUV")
        nc.sync.dma_start(wUV[:], wints_ap[:, 0:1024])

        m7part = outer.tile([128, 128], dt.float32, tag="m7part")
        m8part = outer.tile([128, 32], dt.float32, tag="m8part")
        m9part = outer.tile([128, 64], dt.float32, tag="m9part")

        cc0i = dram.tile([1, 1], dt.float32, tag="cc0i", name="cc0i"); cc0o = dram.tile([1, 1], dt.float32, tag="cc0o", name="cc0o")
        cc1i = dram.tile([1, 1], dt.float32, tag="cc1i", name="cc1i"); cc1o = dram.tile([1, 1], dt.float32, tag="cc1o", name="cc1o")
        cc2i = dram.tile([1, 2], dt.float32, tag="cc2i", name="cc2i"); cc2o = dram.tile([1, 2], dt.float32, tag="cc2o", name="cc2o")
        cc3i = dram.tile([1, 1], dt.float32, tag="cc3i", name="cc3i"); cc3o = dram.tile([1, 1], dt.float32, tag="cc3o", name="cc3o")
        cc4i = dram.tile([1, 1], dt.float32, tag="cc4i", name="cc4i"); cc4o = dram.tile([1, 1], dt.float32, tag="cc4o", name="cc4o")
        cc5i = dram.tile([1, 1], dt.float32, tag="cc5i", name="cc5i"); cc5o = dram.tile([1, 1], dt.float32, tag="cc5o", name="cc5o")

        def allreduce(ci, co_):
            if sim1:
                nc.sync.dma_start(co_[:], ci[:])
            else:
                nc.gpsimd.collective_compute("AllReduce", OP.max, replica_groups=RG,
                                             ins=[ci.opt()], outs=[co_.opt()])

        # column indices in consts
        BQ1, BQ2 = 0, 1
        DW1_0, DW2_0 = 2, 11
        BPW1_0, BPW2, BUP_0 = 20, 24, 25
        CMAG_COL = 27
        cmag = consts[:, CMAG_COL:CMAG_COL + 1]

        big_cm = tc.tile_pool(name="big", bufs=1)
        big = big_cm.__enter__()

        RT = 4  # output rows per psum tile

        def conv3x3_pe(ps_tile, srcs, lhs_fn, y0):
            """3x3 conv tap matmuls into ps_tile [128,RT,W].
            srcs: list of [128,H,W] source views (e.g. hi/lo bf16 pair).
            lhs_fn(tap) -> lhsT AP.  Center tap first (full coverage,
            start=True clears bank; has_written handles partial taps)."""
            order = [4] + [t for t in range(9) if t != 4]
            mms = []
            for t in order:
                dy, dx = TAPS[t]
                ys = max(0, max(0, -dy) - y0)
                ye = min(RT, min(H, H - dy) - y0)
                xs, xe = max(0, -dx), W - max(0, dx)
                if ye <= ys:
                    continue
                for s in srcs:
                    mms.append((t, dy, dx, ys, ye, xs, xe, s))
            for k, (t, dy, dx, ys, ye, xs, xe, s) in enumerate(mms):
                nc.tensor.matmul(ps_tile[:, ys:ye, xs:xe], lhs_fn(t),
                                 s[:, y0 + ys + dy:y0 + ye + dy, xs + dx:xe + dx],
                                 start=(k == 0), stop=(k == len(mms) - 1))

        # =================== P0/P1: load x, dw1, maxes =====================
        xT = big.tile([128, H, W], dt.float32, tag="A")
        nc.sync.dma_start(xT[:, :, :], x_ap[:].rearrange("c (h w) -> c h w", h=H))

        # m3 = max|x| -> collective 0 (overlaps dw1)
        mx3 = sc("mx3loc")
        nc.vector.tensor_reduce(mx3[:], xT[:, :, :], axis=AX.XY, op=OP.max,
                                apply_absolute_value=True)
        cross_max(mx3[:], cc0i, 0)
        allreduce(cc0i, cc0o)
        m3 = bcast(cc0o, 0, "m3")
        s3 = sc("s3"); a3 = sc("a3")
        ts(s3[:], m3[:], float(R127), None, OP.mult)
        nc.vector.reciprocal(a3[:], s3[:])

        sw1, sw2, swp1, swp2, swu = _CACHE["host_scales"]
        NT = NPIX // 512  # 32 n tiles

        # hi/lo bf16 split of x for exact-ish PE depthwise
        xhl = big.tile([128, 2, H, W], dt.bfloat16, tag="B")
        nc.vector.tensor_copy(xhl[:, 0, :, :], xT[:, :, :])
        nc.vector.tensor_tensor(xhl[:, 1, :, :], xT[:, :, :], xhl[:, 0, :, :], OP.subtract)

        pdw_cm = tc.tile_pool(name="pdw", bufs=1)
        pdw = pdw_cm.__enter__()
        wdw = pdw.tile([128, 2 * 9 * 128], dt.bfloat16, tag="wdw", bufs=1)
        nc.sync.dma_start(wdw[:], wints_ap[:, 3328:3328 + 2304])
        dwps_cm = tc.tile_pool(name="dwps", bufs=4, space="PSUM")
        dwps = dwps_cm.__enter__()

        m1part = outer.tile([128, 32], dt.float32, tag="m1part")
        m4part = outer.tile([128, 32], dt.float32, tag="m4part")

        d1 = big.tile([128, H, W], dt.float32, tag="K")  # will hold v1
        for i in range(NT):
            pst = dwps.tile([128, RT, W], dt.float32, tag="dw", name=f"dw1_{i}")
            conv3x3_pe(pst, [xhl[:, 0, :, :], xhl[:, 1, :, :]],
                       lambda t: wdw[:, 128 * t:128 * (t + 1)], RT * i)
            nc.vector.tensor_reduce(m1part[:, i:i + 1], pst[:, :, :], axis=AX.XY, op=OP.max)
            nc.scalar.activation(d1[:, RT * i:RT * (i + 1), :], pst[:, :, :], AF.Relu,
                                 bias=consts[:, BQ1:BQ1 + 1], scale=float(sw1))
        mx1 = sc("mx1loc")
        nc.vector.tensor_reduce(mx1[:], m1part[:, :], axis=AX.X, op=OP.max)
        nc.scalar.activation(mx1[:], mx1[:], AF.Relu, bias=consts[:, BQ1:BQ1 + 1],
                             scale=float(sw1))
        cross_max(mx1[:], cc1i, 0)
        allreduce(cc1i, cc1o)
        m1 = bcast(cc1o, 0, "m1")
        s1 = sc("s1"); a1 = sc("a1"); m1q = sc("m1q"); s2 = sc("s2"); rs2 = sc("rs2"); rho = sc("rho")
        ts(s1[:], m1[:], float(R255), None, OP.mult)
        nc.vector.reciprocal(a1[:], s1[:])
        ts(m1q[:], s1[:], 255.0, None, OP.mult)
        ts(s2[:], m1q[:], float(R127), None, OP.mult)
        nc.vector.reciprocal(rs2[:], s2[:])
        nc.vector.tensor_mul(rho[:], s1[:], rs2[:])

        # =================== P2a: r1 = fq(h1) + fq(x) ======================
        qT = big.tile([128, H, W], dt.float32, tag="B")
        # qx3 = round(x*a3) * s3   (ts#1 on ACT, ts#2 on DVE)
        nc.scalar.activation(qT[:, :, :], xT[:, :, :], AF.Identity,
                             bias=cmag, scale=a3[:])
        ts(qT[:, :, :], qT[:, :, :], CMAGIC, s3[:], OP.subtract, OP.mult)
        # k1' chain in place on d1 (holds v1)
        nc.scalar.activation(d1[:, :, :], d1[:, :, :], AF.Identity,
                             bias=cmag, scale=a1[:])                       # v1*a1 + C
        ts(d1[:, :, :], d1[:, :, :], CMAGIC, rho[:], OP.subtract, OP.mult)  # k1*rho
        ts(d1[:, :, :], d1[:, :, :], CMAGIC, CMAGIC, OP.add, OP.subtract)   # k1'
        # r1 = k1'*s2 + qx3   (in place into qT; gpsimd)
        nc.vector.scalar_tensor_tensor(qT[:, :, :], d1[:, :, :], s2[:], qT[:, :, :],
                                       OP.mult, OP.add)

        # =================== P2b: dw2, m4/m6 ==============================
        mx6 = sc("mx6loc")
        nc.vector.tensor_reduce(mx6[:], qT[:, :, :], axis=AX.XY, op=OP.max,
                                apply_absolute_value=True)
        cross_max(mx6[:], cc2i, 1)
        # hi/lo split of r1
        r1hl = big.tile([128, 2, H, W], dt.bfloat16, tag="A")
        nc.vector.tensor_copy(r1hl[:, 0, :, :], qT[:, :, :])
        nc.vector.tensor_tensor(r1hl[:, 1, :, :], qT[:, :, :], r1hl[:, 0, :, :], OP.subtract)
        d2 = big.tile([128, H, W], dt.float32, tag="K")  # will hold v2
        for i in range(NT):
            pst = dwps.tile([128, RT, W], dt.float32, tag="dw", name=f"dw2_{i}")
            conv3x3_pe(pst, [r1hl[:, 0, :, :], r1hl[:, 1, :, :]],
                       lambda t: wdw[:, 1152 + 128 * t:1152 + 128 * (t + 1)], RT * i)
            nc.vector.tensor_reduce(m4part[:, i:i + 1], pst[:, :, :], axis=AX.XY, op=OP.max)
            nc.scalar.activation(d2[:, RT * i:RT * (i + 1), :], pst[:, :, :], AF.Relu,
                                 bias=consts[:, BQ2:BQ2 + 1], scale=float(sw2))
        mx4 = sc("mx4loc")
        nc.vector.tensor_reduce(mx4[:], m4part[:, :], axis=AX.X, op=OP.max)
        nc.scalar.activation(mx4[:], mx4[:], AF.Relu, bias=consts[:, BQ2:BQ2 + 1],
                             scale=float(sw2))
        cross_max(mx4[:], cc2i, 0)
        allreduce(cc2i, cc2o)
        m4 = bcast(cc2o, 0, "m4"); m6 = bcast(cc2o, 1, "m6")
        s4 = sc("s4"); a4 = sc("a4"); m4q = sc("m4q"); s5 = sc("s5"); rs5 = sc("rs5")
        rho2 = sc("rho2"); s6 = sc("s6"); a6 = sc("a6")
        ts(s4[:], m4[:], float(R255), None, OP.mult)
        nc.vector.reciprocal(a4[:], s4[:])
        ts(m4q[:], s4[:], 255.0, None, OP.mult)
        ts(s5[:], m4q[:], float(R127), None, OP.mult)
        nc.vector.reciprocal(rs5[:], s5[:])
        nc.vector.tensor_mul(rho2[:], s4[:], rs5[:])
        ts(s6[:], m6[:], float(R127), None, OP.mult)
        nc.vector.reciprocal(a6[:], s6[:])

        dwps_cm.__exit__(None, None, None)
        pdw_cm.__exit__(None, None, None)

        # =================== P3a: r2 + hi/lo split ========================
        # k2' chain in place on d2 (holds v2)
        nc.scalar.activation(d2[:, :, :], d2[:, :, :], AF.Identity,
                             bias=cmag, scale=a4[:])
        ts(d2[:, :, :], d2[:, :, :], CMAGIC, rho2[:], OP.subtract, OP.mult)
        ts(d2[:, :, :], d2[:, :, :], CMAGIC, CMAGIC, OP.add, OP.subtract)   # k2'
        # j3 = round(r1*a6)*s6 in place on qT
        nc.scalar.activation(qT[:, :, :], qT[:, :, :], AF.Identity,
                             bias=cmag, scale=a6[:])
        ts(qT[:, :, :], qT[:, :, :], CMAGIC, s6[:], OP.subtract, OP.mult)
        r2T = big.tile([128, NPIX], dt.float32, tag="A")
        nc.vector.scalar_tensor_tensor(r2T[:, :], d2[:, :, :].rearrange("c h w -> c (h w)"),
                                       s5[:], qT[:, :, :].rearrange("c h w -> c (h w)"),
                                       OP.mult, OP.add)
        r2h = big.tile([128, NPIX], dt.bfloat16, tag="K")
        nc.vector.tensor_copy(r2h[:, :], r2T[:, :])
        r2l = big.tile([128, NPIX], dt.bfloat16, tag="B")
        nc.vector.tensor_tensor(r2l[:, :], r2T[:, :], r2h[:, :], OP.subtract)

        # =================== P3b: pw1 pass 1 (max only) ====================
        ps1_cm = tc.tile_pool(name="ps1", bufs=3, space="PSUM")
        ps1 = ps1_cm.__enter__()
        p3_cm = tc.tile_pool(name="p3", bufs=6)
        p3 = p3_cm.__enter__()
        for cchunk in range(4):
            lhs = wUV[:, 128 * cchunk:128 * (cchunk + 1)]
            for i in range(NT):
                ps = ps1.tile([128, 512], dt.float32, tag="pw1", name=f"psA_{cchunk}_{i}")
                nc.tensor.matmul(ps[:], lhs, r2h[:, 512 * i:512 * (i + 1)], start=True, stop=False)
                nc.tensor.matmul(ps[:], lhs, r2l[:, 512 * i:512 * (i + 1)], start=False, stop=True)
                nc.vector.tensor_reduce(m7part[:, cchunk * 32 + i:cchunk * 32 + i + 1],
                                        ps[:], axis=AX.X, op=OP.max)
        mx7c = sc("mx7c"); mx7 = sc("mx7loc")
        for cchunk in range(4):
            nc.vector.tensor_reduce(mx7c[:], m7part[:, 32 * cchunk:32 * (cchunk + 1)],
                                    axis=AX.X, op=OP.max)
            nc.scalar.activation(mx7c[:], mx7c[:], AF.Relu,
                                 bias=consts[:, BPW1_0 + cchunk:BPW1_0 + cchunk + 1],
                                 scale=float(swp1))
            if cchunk == 0:
                nc.vector.tensor_copy(mx7[:], mx7c[:])
            else:
                nc.vector.tensor_tensor(mx7[:], mx7[:], mx7c[:], OP.max)
        cross_max(mx7[:], cc3i, 0)
        allreduce(cc3i, cc3o)
        m7 = bcast(cc3o, 0, "m7")
        s7 = sc("s7"); a7 = sc("a7"); al7 = sc("al7"); alc = sc("alc")
        ts(s7[:], m7[:], float(R255), None, OP.mult)
        nc.vector.reciprocal(a7[:], s7[:])
        ts(al7[:], a7[:], float(swp1), None, OP.mult)
        ts(alc[:], s7[:], float(swp2), None, OP.mult)
        bet7 = []
        for cchunk in range(4):
            b_ = sc(f"bet7_{cchunk}")
            nc.vector.tensor_mul(b_[:], consts[:, BPW1_0 + cchunk:BPW1_0 + cchunk + 1], a7[:])
            bet7.append(b_)

        # =================== P3c: pw1 pass 2 + pw2 =========================
        ps2_cm = tc.tile_pool(name="ps2", bufs=2, space="PSUM")
        ps2 = ps2_cm.__enter__()
        z_cT = big.tile([128, NPIX], dt.float32, tag="A")
        for i in range(NT):
            eqs = []
            for cchunk in range(4):
                lhs = wUV[:, 128 * cchunk:128 * (cchunk + 1)]
                ps = ps1.tile([128, 512], dt.float32, tag="pw1", name=f"psA_{cchunk}_{i}")
                nc.tensor.matmul(ps[:], lhs, r2h[:, 512 * i:512 * (i + 1)], start=True, stop=False)
                nc.tensor.matmul(ps[:], lhs, r2l[:, 512 * i:512 * (i + 1)], start=False, stop=True)
                t_ = p3.tile([128, 512], dt.float32, tag="trelu", name=f"tr_{cchunk}_{i}", bufs=2)
                nc.scalar.activation(t_[:], ps[:], AF.Relu, bias=bet7[cchunk][:], scale=al7[:])
                eq = p3.tile([128, 512], dt.bfloat16, tag="eq", name=f"eq_{cchunk}_{i}", bufs=6)
                ts(eq[:], t_[:], CMAGIC, CMAGIC, OP.add, OP.subtract)
                eqs.append(eq)
            ps2t = ps2.tile([128, 512], dt.float32, tag="pw2", name=f"psB_{i}")
            for cchunk in range(4):
                lhsV = wUV[:, 512 + 128 * cchunk:512 + 128 * (cchunk + 1)]
                nc.tensor.matmul(ps2t[:], lhsV, eqs[cchunk][:],
                                 start=(cchunk == 0), stop=(cchunk == 3))
            nc.vector.tensor_reduce(m8part[:, i:i + 1], ps2t[:], axis=AX.X, op=OP.max)
            nc.scalar.activation(z_cT[:, 512 * i:512 * (i + 1)], ps2t[:], AF.Relu,
                                 bias=consts[:, BPW2:BPW2 + 1], scale=alc[:])
        mx8 = sc("mx8loc")
        nc.vector.tensor_reduce(mx8[:], m8part[:, :], axis=AX.X, op=OP.max)
        nc.scalar.activation(mx8[:], mx8[:], AF.Relu, bias=consts[:, BPW2:BPW2 + 1], scale=alc[:])
        cross_max(mx8[:], cc4i, 0)
        allreduce(cc4i, cc4o)
        m8 = bcast(cc4o, 0, "m8")
        s8 = sc("s8"); a8 = sc("a8"); alu = sc("alu")
        ts(s8[:], m8[:], float(R255), None, OP.mult)
        nc.vector.reciprocal(a8[:], s8[:])
        ts(alu[:], s8[:], float(swu), None, OP.mult)

        # =================== P4: cq =======================================
        uT = big.tile([128, NPIX], dt.float32, tag="B")
        nc.scalar.activation(uT[:, :], z_cT[:, :], AF.Identity, bias=cmag, scale=a8[:])
        cqT = big.tile([128, H, W], dt.bfloat16, tag="K")
        ts(cqT[:, :, :].rearrange("c h w -> c (h w)"), uT[:, :], CMAGIC, None, OP.subtract)

        p3_cm.__exit__(None, None, None)
        ps2_cm.__exit__(None, None, None)
        ps1_cm.__exit__(None, None, None)

        # =================== P5: up conv ===================================
        p5_cm = tc.tile_pool(name="p5", bufs=3)
        p5 = p5_cm.__enter__()
        wup = p5.tile([128, 9 * 256], dt.bfloat16, tag="wup", bufs=1)
        nc.sync.dma_start(wup[:], wints_ap[:, 1024:1024 + 9 * 256])
        ps3_cm = tc.tile_pool(name="ps3", bufs=3, space="PSUM")
        ps3 = ps3_cm.__enter__()
        for cchunk in range(2):
            for i in range(NT):
                y0 = i * RT
                ps = ps3.tile([128, RT, W], dt.float32, tag="up", name=f"psU_{cchunk}_{i}")
                # center tap first: full coverage, start=True clears the bank
                lhs_c = wup[:, 256 * 4 + 128 * cchunk: 256 * 4 + 128 * cchunk + 128]
                nc.tensor.matmul(ps[:, :, :], lhs_c,
                                 cqT[:, y0:y0 + RT, :], start=True, stop=False)
                ntap = 0
                for t, (dy, dx) in enumerate(TAPS):
                    if (dy, dx) == (0, 0):
                        continue
                    ntap += 1
                    ys = max(0, -dy, y0) - y0          # local out row start
                    ye = min(H, H - dy, y0 + RT) - y0  # local out row end
                    xs, xe = max(0, -dx), W - max(0, dx)
                    if ye <= ys:
                        continue
                    lhs = wup[:, 256 * t + 128 * cchunk: 256 * t + 128 * cchunk + 128]
                    nc.tensor.matmul(ps[:, ys:ye, xs:xe], lhs,
                                     cqT[:, y0 + ys + dy:y0 + ye + dy, xs + dx:xe + dx],
                                     start=False, stop=(ntap == 8))
                nc.vector.tensor_reduce(m9part[:, cchunk * 32 + i:cchunk * 32 + i + 1],
                                        ps[:, :, :], axis=AX.XY, op=OP.max)
                zt = p5.tile([128, RT * W], dt.float32, tag="zt", name=f"zt_{cchunk}_{i}")
                nc.scalar.activation(zt[:], ps[:, :, :].rearrange("c r w -> c (r w)"),
                                     AF.Relu, bias=consts[:, BUP_0 + cchunk:BUP_0 + cchunk + 1],
                                     scale=alu[:])
                nc.sync.dma_start(
                    zdram.ap()[128 * cchunk:128 * (cchunk + 1), 512 * i:512 * (i + 1)], zt[:])
        mx9c = sc("mx9c"); mx9 = sc("mx9loc")
        for cchunk in range(2):
            nc.vector.tensor_reduce(mx9c[:], m9part[:, 32 * cchunk:32 * (cchunk + 1)],
                                    axis=AX.X, op=OP.max)
            nc.scalar.activation(mx9c[:], mx9c[:], AF.Relu,
                                 bias=consts[:, BUP_0 + cchunk:BUP_0 + cchunk + 1],
                                 scale=alu[:])
            if cchunk == 0:
                nc.vector.tensor_copy(mx9[:], mx9c[:])
            else:
                nc.vector.tensor_tensor(mx9[:], mx9[:], mx9c[:], OP.max)
        cross_max(mx9[:], cc5i, 0)
        allreduce(cc5i, cc5o)
        m9 = bcast(cc5o, 0, "m9")
        s9 = sc("s9"); a9 = sc("a9")
        ts(s9[:], m9[:], float(R255), None, OP.mult)
        nc.vector.reciprocal(a9[:], s9[:])

        ps3_cm.__exit__(None, None, None)
        p5_cm.__exit__(None, None, None)
        big_cm.__exit__(None, None, None)

        # =================== P6: final quantize + store ====================
        p6_cm = tc.tile_pool(name="p6", bufs=3)
        p6 = p6_cm.__enter__()
        FCOL = 2048
        for cchunk in range(2):
            for f in range(NPIX // FCOL):
                ft = p6.tile([128, FCOL], dt.float32, tag="fin", name=f"fin_{cchunk}_{f}")
                nc.sync.dma_start(ft[:], zdram.ap()[128 * cchunk:128 * (cchunk + 1),
                                                    FCOL * f:FCOL * (f + 1)])
                nc.scalar.activation(ft[:], ft[:], AF.Identity, bias=cmag, scale=a9[:])
                ts(ft[:], ft[:], CMAGIC, s9[:], OP.subtract, OP.mult)
                nc.sync.dma_start(out_ap[128 * cchunk:128 * (cchunk + 1),
                                         FCOL * f:FCOL * (f + 1)], ft[:])
        p6_cm.__exit__(None, None, None)
        dram_cm.__exit__(None, None, None)
        outer_cm.__exit__(None, None, None)

    nc.compile()
    _CACHE["nc"] = nc
    return _CACHE


def _prep_host(inputs):
    """Host-side exact weight fake-quant + packing. Returns (wints, consts, scales)."""
    q1, sdw1 = _fq_int(inputs["dw1_w"])     # (128,1,3,3)
    qb1, sb1 = _fq_int(inputs["dw1_b"])
    q2, sdw2 = _fq_int(inputs["dw2_w"])
    qb2, sb2 = _fq_int(inputs["dw2_b"])
    qp1, sp1 = _fq_int(inputs["pw1_w"])     # (512,128,1,1)
    qbp1, sbp1 = _fq_int(inputs["pw1_b"])
    qp2, sp2 = _fq_int(inputs["pw2_w"])     # (128,512,1,1)
    qbp2, sbp2 = _fq_int(inputs["pw2_b"])
    qu, su = _fq_int(inputs["up_w"])        # (256,128,3,3)
    qbu, sbu = _fq_int(inputs["up_b"])

    # consts [128, 27] fp32
    consts = np.zeros((128, 28), np.float32)
    consts[:, 27] = np.float32(12582912.0)
    consts[:, 0] = (qb1 * sb1).astype(np.float32)
    consts[:, 1] = (qb2 * sb2).astype(np.float32)
    w1v = (q1 * sdw1).astype(np.float32)    # actual fp32 quantized weight values
    w2v = (q2 * sdw2).astype(np.float32)
    for t in range(9):
        ky, kx = t // 3, t % 3
        consts[:, 2 + t] = w1v[:, 0, ky, kx]
        consts[:, 11 + t] = w2v[:, 0, ky, kx]
    bp1v = (qbp1 * sbp1).astype(np.float32)
    for cchunk in range(4):
        consts[:, 20 + cchunk] = bp1v[128 * cchunk:128 * (cchunk + 1)]
    consts[:, 24] = (qbp2 * sbp2).astype(np.float32)
    bupv = (qbu * sbu).astype(np.float32)
    consts[:, 25] = bupv[0:128]
    consts[:, 26] = bupv[128:256]

    # wints [128, 1024 + 2304 + 2304] bf16
    wints = np.zeros((128, 512 + 512 + 9 * 256 + 2304), ml_dtypes.bfloat16)
    wints[:, 0:512] = qp1[:, :, 0, 0].T.astype(ml_dtypes.bfloat16)      # U lhsT [ci, co]
    V = qp2[:, :, 0, 0]                                                  # (128, 512)
    for cchunk in range(4):
        wints[:, 512 + 128 * cchunk:512 + 128 * (cchunk + 1)] = \
            V[:, 128 * cchunk:128 * (cchunk + 1)].T.astype(ml_dtypes.bfloat16)
    for t in range(9):
        ky, kx = t // 3, t % 3
        wints[:, 1024 + 256 * t:1024 + 256 * (t + 1)] = \
            qu[:, :, ky, kx].T.astype(ml_dtypes.bfloat16)
    for t in range(9):
        ky, kx = t // 3, t % 3
        wints[:, 3328 + 128 * t:3328 + 128 * (t + 1)] = \
            np.diag(q1[:, 0, ky, kx]).astype(ml_dtypes.bfloat16)
        wints[:, 3328 + 1152 + 128 * t:3328 + 1152 + 128 * (t + 1)] = \
            np.diag(q2[:, 0, ky, kx]).astype(ml_dtypes.bfloat16)
    scales = (float(sdw1), float(sdw2), float(sp1), float(sp2), float(su))
    return wints, consts, scales


def _get_runner():
    """Persistent jitted 8-core executor (mirrors bass2jax.run_bass_via_pjrt,
    but caches the jitted callable so repeat calls don't re-trace)."""
    if "runner" in _CACHE:
        return _CACHE["runner"]
    import jax
    from jax.sharding import Mesh, PartitionSpec
    from jax.experimental.shard_map import shard_map
    from concourse import bass2jax

    nc = _CACHE["nc"]
    bass2jax.install_neuronx_cc_hook()
    partition_name = nc.partition_id_tensor.name if nc.partition_id_tensor else None
    in_names, out_names, out_avals, zero_outs = [], [], [], []
    for alloc in nc.m.functions[0].allocations:
        if not isinstance(alloc, mybir.MemoryLocationSet):
            continue
        name = alloc.memorylocations[0].name
        if alloc.kind == "ExternalInput":
            if name != partition_name:
                in_names.append(name)
        elif alloc.kind == "ExternalOutput":
            shape = tuple(alloc.tensor_shape)
            dtype = mybir.dt.np(alloc.dtype)
            out_names.append(name)
            out_avals.append(jax.core.ShapedArray(shape, dtype))
            zero_outs.append(np.zeros(shape, dtype))
    n_params = len(in_names)
    all_in_names = list(in_names) + list(out_names)
    if partition_name is not None:
        all_in_names.append(partition_name)

    def _body(*args):
        operands = list(args)
        if partition_name is not None:
            operands.append(bass2jax.partition_id_tensor())
        outs = bass2jax._bass_exec_p.bind(
            *operands,
            out_avals=tuple(out_avals),
            in_names=tuple(all_in_names),
            out_names=tuple(out_names),
            lowering_input_output_aliases=(),
            sim_require_finite=True,
            sim_require_nnan=True,
            nc=nc,
        )
        return tuple(outs)

    devices = jax.devices()[:N_CORES]
    mesh = Mesh(np.asarray(devices), ("core",))
    n_outs = len(out_names)
    in_specs = (PartitionSpec("core"),) * (n_params + n_outs)
    out_specs = (PartitionSpec("core"),) * n_outs
    sharded = jax.jit(
        shard_map(_body, mesh=mesh, in_specs=in_specs, out_specs=out_specs,
                  check_rep=False),
        keep_unused=True,
    )
    concat_zeros = [
        jax.device_put(np.zeros((N_CORES * z.shape[0], *z.shape[1:]), z.dtype))
        for z in zero_outs
    ]

    def run(concat_inputs):
        """concat_inputs: dict name -> (8*dim0, ...) arrays (np or jax)."""
        args = [concat_inputs[n] for n in in_names]
        outs = sharded(*args, *concat_zeros)
        return {name: outs[i] for i, name in enumerate(out_names)}

    _CACHE["runner"] = (run, in_names, out_names)
    return _CACHE["runner"]


def _prep_concat(inputs):
    wints, consts, scales = _prep_host(inputs)
    if "host_scales" in _CACHE:
        assert _CACHE["host_scales"] == scales, "weight scales changed; rebuild needed"
    _CACHE["host_scales"] = scales
    _build()
    x = np.asarray(inputs["x"], np.float32)
    concat = {
        "x": np.ascontiguousarray(x.reshape(N_CORES * C, NPIX)),
        "wints": np.concatenate([wints] * N_CORES, axis=0),
        "consts": np.concatenate([consts] * N_CORES, axis=0),
    }
    return concat


def kernel(**inputs):
    concat = _prep_concat(inputs)
    run, in_names, out_names = _get_runner()
    outs = run(concat)
    out = np.asarray(outs["out"]).reshape(N_CORES, CO, H, W)
    return out.astype(np.float32)
